# revision 26
# baseline (speedup 1.0000x reference)
"""AV temporal gated-conv MLP block for Trainium2 (8 NeuronCores, Bass/Tile).

Per-core strategy: pure data parallelism over the batch (B=8 -> 1 batch
element per core, both modalities on the same core since the gating couples
them). No collectives. Per core, loop over 4 l-blocks of 512 tokens:

  T: rms-norm in natural [l, d] layout (ACT Square+accum_out -> Sqrt -> DVE
     reciprocal -> tensor_scalar scale), then PE-transpose x_n to [d, l].
  A: in_proj as fp32r matmuls producing [e, l] chunks; causal depthwise conv
     on the x-half via 4 diagonal matmuls + K=1 bias matmul accumulated in
     PSUM (diag built on the fly: identity x per-partition conv weight);
     silu (ACT) on the w-half straight out of PSUM; cross-modal gate on DVE.
  B: out_proj fp32r matmuls [l, d] + residual add + DMA out.

Weights are host-side packed: norm weights folded into in_proj, transposed to
[d, e] / [e, d] m-tile-contiguous layouts so every weight DMA is contiguous.
"""
import sys

if "/opt/trn_rl_repo" not in sys.path:
    sys.path.insert(0, "/opt/trn_rl_repo")

import numpy as np

DIM = 1024
INNER = 2048
L = 2048
B = 8
NCORES = 8
EPS = 1e-5
LB = 512              # l-block (tokens per block)
NB = L // LB          # 4 blocks
NXC = INNER // 128    # 16 x-half e-chunks per modality
NKD = DIM // 128      # 8 contraction chunks for in_proj
NLT = LB // 128       # 4 l-tiles per block
NN = DIM // 512       # 2 out_proj n-tiles
CONV_MODE = "dve"  # "pe" | "dve" | "gps" | "dve+gps" | "pe+dve"

_cache = {}


def _build_nc(conv_mode=None, repeat=1):
    conv_mode = conv_mode or CONV_MODE
    from contextlib import ExitStack

    import concourse.bass as bass
    import concourse.tile as tile
    from concourse import bacc, mybir
    from concourse.masks import make_identity

    dt = mybir.dt
    f32 = dt.float32
    f32r = dt.float32r
    bf16 = dt.bfloat16
    AOP = mybir.AluOpType
    AF = mybir.ActivationFunctionType

    nc = bacc.Bacc("TRN2", target_bir_lowering=False, debug=False,
                   num_devices=NCORES)

    x_dram = {
        "a": nc.dram_tensor("xa", [L, DIM], f32, kind="ExternalInput").ap(),
        "v": nc.dram_tensor("xv", [L, DIM], f32, kind="ExternalInput").ap(),
    }
    win_dram = {
        "a": nc.dram_tensor("wina", [2 * NXC, 128, NKD, 128], f32r,
                            kind="ExternalInput").ap(),
        "v": nc.dram_tensor("winv", [2 * NXC, 128, NKD, 128], f32r,
                            kind="ExternalInput").ap(),
    }
    wout_dram = {
        "a": nc.dram_tensor("wouta", [NXC, NN, 128, 512], f32r,
                            kind="ExternalInput").ap(),
        "v": nc.dram_tensor("woutv", [NXC, NN, 128, 512], f32r,
                            kind="ExternalInput").ap(),
    }
    cw_dram = {
        "a": nc.dram_tensor("cwa", [128, NXC * 4], f32, kind="ExternalInput").ap(),
        "v": nc.dram_tensor("cwv", [128, NXC * 4], f32, kind="ExternalInput").ap(),
    }
    cbc_dram = {
        "a": nc.dram_tensor("cbca", [128, NXC], f32, kind="ExternalInput").ap(),
        "v": nc.dram_tensor("cbcv", [128, NXC], f32, kind="ExternalInput").ap(),
    }
    y = nc.dram_tensor("y", [2 * L, DIM], f32, kind="ExternalOutput").ap()

    MODS = ("a", "v")

    with tile.TileContext(nc) as tc, ExitStack() as ctx:
        sing = ctx.enter_context(tc.tile_pool(name="sing", bufs=1))
        p_xT = ctx.enter_context(tc.tile_pool(name="xT", bufs=2))
        p_gat = ctx.enter_context(tc.tile_pool(name="gat", bufs=2))
        p_xin = ctx.enter_context(tc.tile_pool(name="xin", bufs=3))
        p_xn = ctx.enter_context(tc.tile_pool(name="xn", bufs=3))
        p_stat = ctx.enter_context(tc.tile_pool(name="stat", bufs=8))
        p_win = ctx.enter_context(tc.tile_pool(name="win", bufs=6))
        p_wout = ctx.enter_context(tc.tile_pool(name="wout", bufs=4))
        p_axp = ctx.enter_context(tc.tile_pool(name="axp", bufs=4))
        p_sv = ctx.enter_context(tc.tile_pool(name="sv", bufs=3))
        p_diag = ctx.enter_context(tc.tile_pool(name="diag", bufs=4))
        p_res = ctx.enter_context(tc.tile_pool(name="res", bufs=2))
        p_yout = ctx.enter_context(tc.tile_pool(name="yout", bufs=2))
        p_ps = ctx.enter_context(
            tc.tile_pool(name="ps", bufs=8, space=bass.MemorySpace.PSUM))

        ident_f32 = sing.tile([128, 128], f32, name="ident_f32", tag="ident_f32")
        make_identity(nc, ident_f32[:])
        identity = sing.tile([128, 128], f32r, name="identity", tag="identity")
        nc.vector.tensor_copy(identity[:], ident_f32[:])
        epscol = sing.tile([128, 1], f32, name="epscol", tag="epscol")
        nc.vector.memset(epscol[:], EPS)

        cw_sb, cbc_sb, hist = {}, {}, {}
        for mod in MODS:
            cw_sb[mod] = sing.tile([128, NXC * 4], f32, name=f"cw_{mod}", tag=f"cw_{mod}")
            nc.sync.dma_start(cw_sb[mod][:], cw_dram[mod][:])
            cbc_sb[mod] = sing.tile([128, NXC], f32, name=f"cbc_{mod}", tag=f"cbc_{mod}")
            nc.sync.dma_start(cbc_sb[mod][:], cbc_dram[mod][:])
            hist[mod] = sing.tile([128, NXC * 3], f32, name=f"hist_{mod}", tag=f"hist_{mod}")
            nc.vector.memset(hist[mod][:], 0.0)

        def emit_T(blk):
            l0 = blk * LB
            xTt = {}
            for mod in MODS:
                xTt[mod] = p_xT.tile([128, NKD * LB], f32r, name="xT", tag="xT")
                for lt in range(NLT):
                    xt = p_xin.tile([128, DIM], f32, name="xin", tag="xin")
                    nc.sync.dma_start(
                        xt[:], x_dram[mod][l0 + lt * 128: l0 + (lt + 1) * 128, :])
                    xnt = p_xn.tile([128, DIM], f32r, name="xn", tag="xn")
                    ssum = p_stat.tile([128, 1], f32, name="ssum", tag="ssum")
                    nc.scalar.activation(xnt[:], xt[:], AF.Square,
                                         accum_out=ssum[:])
                    sroot = p_stat.tile([128, 1], f32, name="sroot", tag="sroot")
                    nc.scalar.activation(sroot[:], ssum[:], AF.Sqrt,
                                         bias=epscol[:], scale=1.0 / DIM)
                    sinv = p_stat.tile([128, 1], f32, name="sinv", tag="sinv")
                    nc.vector.reciprocal(sinv[:], sroot[:])
                    nc.vector.tensor_scalar(xnt[:], xt[:], sinv[:], None,
                                            AOP.mult)
                    for g in range(2):
                        pt = p_ps.tile([128, 512], f32, name="ps", tag="ps")
                        for j in range(4):
                            dc = g * 4 + j
                            nc.tensor.matmul(
                                pt[:, j * 128:(j + 1) * 128].bitcast(f32r),
                                lhsT=xnt[:, dc * 128:(dc + 1) * 128],
                                rhs=identity[:],
                                is_transpose=True, skip_group_check=True)
                        dst = xTt[mod].rearrange("p (dc l) -> p dc l", dc=NKD)[
                            :, g * 4:(g + 1) * 4, lt * 128:(lt + 1) * 128]
                        src = pt[:].rearrange("p (j l) -> p j l", j=4)
                        if g == 0:
                            nc.vector.tensor_copy(dst, src)
                        else:
                            nc.scalar.copy(dst, src)

            return xTt

        def prefetch_w(blk):
            tiles = {}
            for mod in MODS:
                for half, m in (("x", 0), ("w", NXC)):
                    wt = p_win.tile([128, DIM], f32r, name="win", tag="win")
                    nc.sync.dma_start(
                        wt[:].rearrange("p (kc m) -> p kc m", kc=NKD),
                        win_dram[mod][m])
                    tiles[(mod, half)] = wt
            return tiles

        def emit_A(blk, xTt, pre=None):
            gat = {}
            for mod in MODS:
                gat[mod] = p_gat.tile([128, NXC * LB], f32r, name="gat", tag="gat")
            for c in range(NXC):
                pp = {}
                for mod in MODS:
                    for half, m in (("x", c), ("w", NXC + c)):
                        if c == 0 and pre is not None:
                            wt = pre[(mod, half)]
                        else:
                            wt = p_win.tile([128, DIM], f32r, name="win", tag="win")
                            nc.sync.dma_start(
                                wt[:].rearrange("p (kc m) -> p kc m", kc=NKD),
                                win_dram[mod][m])
                        ps = p_ps.tile([128, LB], f32, name="ps", tag="ps")
                        for kc in range(NKD):
                            nc.tensor.matmul(
                                ps[:],
                                lhsT=wt[:, kc * 128:(kc + 1) * 128],
                                rhs=xTt[mod][:, kc * LB:(kc + 1) * LB],
                                start=(kc == 0), stop=(kc == NKD - 1))
                        pp[(mod, half)] = ps
                sv = {}
                for mod in MODS:
                    s = p_sv.tile([128, LB], f32, name="sv", tag="sv")
                    nc.scalar.activation(s[:], pp[(mod, "w")][:], AF.Silu)
                    sv[mod] = s
                for mi, (mod, other) in enumerate((("a", "v"), ("v", "a"))):
                    if conv_mode == "pe":
                        cm = "pe"
                    elif conv_mode == "dve":
                        cm = "dve"
                    elif conv_mode == "gps":
                        cm = "gps"
                    elif conv_mode == "dve+gps":
                        cm = "dve" if mi == 0 else "gps"
                    elif conv_mode == "pe+dve":
                        cm = "pe" if mi == 0 else "dve"
                    else:
                        raise ValueError(conv_mode)
                    axp = p_axp.tile([128, LB + 3], f32r, name="axp", tag="axp")
                    nc.vector.tensor_copy(axp[:, 0:3],
                                          hist[mod][:, c * 3:(c + 1) * 3])
                    nc.scalar.copy(axp[:, 3:LB + 3], pp[(mod, "x")][:])
                    nc.vector.tensor_copy(hist[mod][:, c * 3:(c + 1) * 3],
                                          axp[:, LB:LB + 3])
                    if cm == "pe":
                        cp = p_ps.tile([128, LB], f32, name="ps", tag="ps")
                        for t in range(4):
                            dg = p_diag.tile([128, 128], f32r, name="diag", tag="diag")
                            nc.vector.tensor_scalar(
                                dg[:], identity[:],
                                cw_sb[mod][:, c * 4 + t: c * 4 + t + 1],
                                None, AOP.mult)
                            nc.tensor.matmul(cp[:], lhsT=dg[:],
                                             rhs=axp[:, t:t + LB],
                                             start=(t == 0), stop=(t == 3))
                        conv_src = cp
                    else:
                        eng = nc.vector if cm == "dve" else nc.gpsimd
                        acc = p_sv.tile([128, LB], f32, name="convacc", tag="convacc")
                        eng.tensor_scalar(
                            acc[:], axp[:, 0:LB],
                            cw_sb[mod][:, c * 4: c * 4 + 1],
                            cbc_sb[mod][:, c:c + 1], AOP.mult, AOP.add)
                        for t in range(1, 4):
                            acc2 = p_sv.tile([128, LB], f32, name="convacc", tag="convacc")
                            eng.scalar_tensor_tensor(
                                acc2[:], axp[:, t:t + LB],
                                cw_sb[mod][:, c * 4 + t: c * 4 + t + 1],
                                acc[:], AOP.mult, AOP.add)
                            acc = acc2
                        conv_src = acc
                    if cm == "pe":
                        nc.vector.scalar_tensor_tensor(
                            gat[mod][:, c * LB:(c + 1) * LB], conv_src[:],
                            cbc_sb[mod][:, c:c + 1], sv[other][:],
                            AOP.add, AOP.mult)
                    else:
                        nc.vector.tensor_mul(gat[mod][:, c * LB:(c + 1) * LB],
                                             conv_src[:], sv[other][:])

            return gat

        def emit_B(blk, gat):
            l0 = blk * LB
            for mod in MODS:
                yoff = 0 if mod == "a" else L
                for n in range(NN):
                    po = [p_ps.tile([128, 512], f32, name="ps", tag="ps")
                          for _ in range(NLT)]
                    for c2 in range(NXC):
                        w = p_wout.tile([128, 512], f32r, name="wout", tag="wout")
                        nc.sync.dma_start(w[:], wout_dram[mod][c2, n])
                        for mt in range(NLT):
                            nc.tensor.matmul(
                                po[mt][:],
                                lhsT=gat[mod][:, c2 * LB + mt * 128:
                                              c2 * LB + (mt + 1) * 128],
                                rhs=w[:],
                                start=(c2 == 0), stop=(c2 == NXC - 1))
                    rt = p_res.tile([128, NLT * 512], f32, name="res", tag="res")
                    nc.sync.dma_start(
                        rt[:].rearrange("p (mt d) -> p mt d", mt=NLT),
                        x_dram[mod][l0: l0 + LB, n * 512:(n + 1) * 512]
                        .rearrange("(mt p) d -> p mt d", p=128))
                    yt = p_yout.tile([128, NLT * 512], f32, name="yout", tag="yout")
                    for mt in range(NLT):
                        nc.vector.tensor_add(yt[:, mt * 512:(mt + 1) * 512],
                                             po[mt][:], rt[:, mt * 512:(mt + 1) * 512])
                    nc.sync.dma_start(
                        y[yoff + l0: yoff + l0 + LB, n * 512:(n + 1) * 512]
                        .rearrange("(mt p) d -> p mt d", p=128),
                        yt[:].rearrange("p (mt d) -> p mt d", mt=NLT))

        for rep in range(repeat):
            pre = None
            for blk in range(NB):
                xTt_cur = emit_T(blk)
                gat = emit_A(blk, xTt_cur, pre)
                pre = prefetch_w(blk + 1) if blk + 1 < NB else None
                emit_B(blk, gat)

    nc.finalize()
    return nc


def _get_nc(conv_mode=None, repeat=1):
    key = ("nc", conv_mode or CONV_MODE, repeat)
    if key not in _cache:
        _cache[key] = _build_nc(conv_mode, repeat)
    return _cache[key]


def _prep_weights(inputs):
    f = np.float32
    a_in = np.asarray(inputs["a_in_w"], f) * np.asarray(inputs["a_norm_w"], f)[None, :]
    v_in = np.asarray(inputs["v_in_w"], f) * np.asarray(inputs["v_norm_w"], f)[None, :]

    def pack_in(w):  # [2*INNER, DIM] -> [32, 128, 8, 128]: m-tile x [d%128, d//128, e]
        t = w.T.reshape(NKD, 128, 2 * NXC, 128)   # [kc, p, m, e]
        return np.ascontiguousarray(t.transpose(2, 1, 0, 3))

    def pack_out(w):  # [DIM, INNER] -> [16, 2, 128, 512] (e-chunk, d-tile)
        return np.ascontiguousarray(
            w.T.reshape(NXC, 128, NN, 512).transpose(0, 2, 1, 3))

    def pack_cw(w):  # [INNER, 1, 4] -> [128, 64]
        return np.ascontiguousarray(
            np.asarray(w, f)[:, 0, :].reshape(NXC, 128, 4)
            .transpose(1, 0, 2).reshape(128, NXC * 4))

    return {
        "wina": pack_in(a_in),
        "winv": pack_in(v_in),
        "wouta": pack_out(np.asarray(inputs["a_out_w"], f)),
        "woutv": pack_out(np.asarray(inputs["v_out_w"], f)),
        "cwa": pack_cw(inputs["a_conv_w"]),
        "cwv": pack_cw(inputs["v_conv_w"]),
        "cbca": np.ascontiguousarray(
            np.asarray(inputs["a_conv_b"], f).reshape(NXC, 128).T),
        "cbcv": np.ascontiguousarray(
            np.asarray(inputs["v_conv_b"], f).reshape(NXC, 128).T),
    }


def _make_in_maps(inputs):
    shared = _prep_weights(inputs)
    audio = np.ascontiguousarray(np.asarray(inputs["audio"], np.float32))
    visual = np.ascontiguousarray(np.asarray(inputs["visual"], np.float32))
    in_maps = []
    for b in range(B):
        m = dict(shared)
        m["xa"] = audio[b]
        m["xv"] = visual[b]
        in_maps.append(m)
    return in_maps


def _run(in_maps):
    from concourse.bass_utils import run_bass_kernel_spmd
    nc = _get_nc()
    res = run_bass_kernel_spmd(nc, in_maps, core_ids=list(range(NCORES)))
    return np.stack([res.results[b]["y"] for b in range(B)], axis=0)


def kernel(**inputs) -> np.ndarray:
    return _run(_make_in_maps(inputs))


# revision 29
# speedup vs baseline: 1.9069x; 1.9069x over previous
"""AV temporal gated-conv MLP block for Trainium2 (8 NeuronCores, Bass/Tile).

Per-core strategy: pure data parallelism over the batch (B=8 -> 1 batch
element per core, both modalities on the same core since the gating couples
them). No collectives. Per core, loop over 4 l-blocks of 512 tokens:

  T: rms-norm in natural [l, d] layout (ACT Square+accum_out -> Sqrt -> DVE
     reciprocal -> tensor_scalar scale), then PE-transpose x_n to [d, l].
  A: in_proj as fp32r matmuls producing [e, l] chunks; causal depthwise conv
     on the x-half via 4 diagonal matmuls + K=1 bias matmul accumulated in
     PSUM (diag built on the fly: identity x per-partition conv weight);
     silu (ACT) on the w-half straight out of PSUM; cross-modal gate on DVE.
  B: out_proj fp32r matmuls [l, d] + residual add + DMA out.

Weights are host-side packed: norm weights folded into in_proj, transposed to
[d, e] / [e, d] m-tile-contiguous layouts so every weight DMA is contiguous.
"""
import sys

if "/opt/trn_rl_repo" not in sys.path:
    sys.path.insert(0, "/opt/trn_rl_repo")

import numpy as np

DIM = 1024
INNER = 2048
L = 2048
B = 8
NCORES = 8
EPS = 1e-5
LB = 512              # l-block (tokens per block)
NB = L // LB          # 4 blocks
NXC = INNER // 128    # 16 x-half e-chunks per modality
NKD = DIM // 128      # 8 contraction chunks for in_proj
NLT = LB // 128       # 4 l-tiles per block
NN = DIM // 512       # 2 out_proj n-tiles
CONV_MODE = "dve"  # "pe" | "dve" | "gps" | "dve+gps" | "pe+dve"

_cache = {}


def _build_nc(conv_mode=None, repeat=1):
    conv_mode = conv_mode or CONV_MODE
    from contextlib import ExitStack

    import concourse.bass as bass
    import concourse.tile as tile
    from concourse import bacc, mybir
    from concourse.masks import make_identity

    dt = mybir.dt
    f32 = dt.float32
    f32r = dt.float32r
    bf16 = dt.bfloat16
    AOP = mybir.AluOpType
    AF = mybir.ActivationFunctionType

    nc = bacc.Bacc("TRN2", target_bir_lowering=False, debug=False,
                   num_devices=NCORES)

    x_dram = {
        "a": nc.dram_tensor("xa", [L, DIM], f32, kind="ExternalInput").ap(),
        "v": nc.dram_tensor("xv", [L, DIM], f32, kind="ExternalInput").ap(),
    }
    win_dram = {
        "a": nc.dram_tensor("wina", [2 * NXC, 128, NKD, 128], f32r,
                            kind="ExternalInput").ap(),
        "v": nc.dram_tensor("winv", [2 * NXC, 128, NKD, 128], f32r,
                            kind="ExternalInput").ap(),
    }
    wout_dram = {
        "a": nc.dram_tensor("wouta", [NXC, NN, 128, 512], f32r,
                            kind="ExternalInput").ap(),
        "v": nc.dram_tensor("woutv", [NXC, NN, 128, 512], f32r,
                            kind="ExternalInput").ap(),
    }
    cw_dram = {
        "a": nc.dram_tensor("cwa", [128, NXC * 4], f32, kind="ExternalInput").ap(),
        "v": nc.dram_tensor("cwv", [128, NXC * 4], f32, kind="ExternalInput").ap(),
    }
    cbc_dram = {
        "a": nc.dram_tensor("cbca", [128, NXC], f32, kind="ExternalInput").ap(),
        "v": nc.dram_tensor("cbcv", [128, NXC], f32, kind="ExternalInput").ap(),
    }
    y = nc.dram_tensor("y", [2 * L, DIM], f32, kind="ExternalOutput").ap()

    MODS = ("a", "v")

    with tile.TileContext(nc) as tc, ExitStack() as ctx:
        sing = ctx.enter_context(tc.tile_pool(name="sing", bufs=1))
        p_xT = ctx.enter_context(tc.tile_pool(name="xT", bufs=2))
        p_gat = ctx.enter_context(tc.tile_pool(name="gat", bufs=2))
        p_xin = ctx.enter_context(tc.tile_pool(name="xin", bufs=3))
        p_xn = ctx.enter_context(tc.tile_pool(name="xn", bufs=5))
        p_stat = ctx.enter_context(tc.tile_pool(name="stat", bufs=8))
        p_win = ctx.enter_context(tc.tile_pool(name="win", bufs=6))
        p_wout = ctx.enter_context(tc.tile_pool(name="wout", bufs=4))
        p_axp = ctx.enter_context(tc.tile_pool(name="axp", bufs=4))
        p_sv = ctx.enter_context(tc.tile_pool(name="sv", bufs=3))
        p_diag = ctx.enter_context(tc.tile_pool(name="diag", bufs=4))
        p_res = ctx.enter_context(tc.tile_pool(name="res", bufs=4))
        p_yout = ctx.enter_context(tc.tile_pool(name="yout", bufs=4))
        p_ps = ctx.enter_context(
            tc.tile_pool(name="ps", bufs=8, space=bass.MemorySpace.PSUM))

        ident_f32 = sing.tile([128, 128], f32, name="ident_f32", tag="ident_f32")
        make_identity(nc, ident_f32[:])
        identity = sing.tile([128, 128], f32r, name="identity", tag="identity")
        nc.vector.tensor_copy(identity[:], ident_f32[:])
        epscol = sing.tile([128, 1], f32, name="epscol", tag="epscol")
        nc.vector.memset(epscol[:], EPS)

        cw_sb, cbc_sb, hist = {}, {}, {}

        def setup_conv_state():
            for mod in MODS:
                cw_sb[mod] = sing.tile([128, NXC * 4], f32, name=f"cw_{mod}",
                                       tag=f"cw_{mod}")
                nc.sync.dma_start(cw_sb[mod][:], cw_dram[mod][:])
                cbc_sb[mod] = sing.tile([128, NXC], f32, name=f"cbc_{mod}",
                                        tag=f"cbc_{mod}")
                nc.sync.dma_start(cbc_sb[mod][:], cbc_dram[mod][:])
                hist[mod] = sing.tile([128, NXC * 3], f32, name=f"hist_{mod}",
                                      tag=f"hist_{mod}")
                nc.vector.memset(hist[mod][:], 0.0)

        def emit_T(blk):
            l0 = blk * LB
            xTt = {}
            for mod in MODS:
                xTt[mod] = p_xT.tile([128, NKD * LB], f32r, name="xT", tag="xT")
                for lt in range(NLT):
                    xt = p_xin.tile([128, DIM], f32, name="xin", tag="xin")
                    nc.sync.dma_start(
                        xt[:], x_dram[mod][l0 + lt * 128: l0 + (lt + 1) * 128, :])
                    xnt = p_xn.tile([128, DIM], f32r, name="xn", tag="xn")
                    ssum = p_stat.tile([128, 1], f32, name="ssum", tag="ssum")
                    nc.scalar.activation(xnt[:], xt[:], AF.Square,
                                         accum_out=ssum[:])
                    sroot = p_stat.tile([128, 1], f32, name="sroot", tag="sroot")
                    nc.scalar.activation(sroot[:], ssum[:], AF.Sqrt,
                                         bias=epscol[:], scale=1.0 / DIM)
                    sinv = p_stat.tile([128, 1], f32, name="sinv", tag="sinv")
                    nc.vector.reciprocal(sinv[:], sroot[:])
                    nc.vector.tensor_scalar(xnt[:], xt[:], sinv[:], None,
                                            AOP.mult)
                    for g in range(2):
                        pt = p_ps.tile([128, 512], f32, name="ps", tag="ps")
                        for j in range(4):
                            dc = g * 4 + j
                            nc.tensor.matmul(
                                pt[:, j * 128:(j + 1) * 128].bitcast(f32r),
                                lhsT=xnt[:, dc * 128:(dc + 1) * 128],
                                rhs=identity[:],
                                is_transpose=True, skip_group_check=True)
                        dst = xTt[mod].rearrange("p (dc l) -> p dc l", dc=NKD)[
                            :, g * 4:(g + 1) * 4, lt * 128:(lt + 1) * 128]
                        src = pt[:].rearrange("p (j l) -> p j l", j=4)
                        if g == 0:
                            nc.vector.tensor_copy(dst, src)
                        else:
                            nc.scalar.copy(dst, src)

            return xTt

        def prefetch_w(blk):
            tiles = {}
            for mod in MODS:
                for half, m in (("x", 0), ("w", NXC)):
                    wt = p_win.tile([128, DIM], f32r, name="win", tag="win")
                    nc.sync.dma_start(
                        wt[:].rearrange("p (kc m) -> p kc m", kc=NKD),
                        win_dram[mod][m])
                    tiles[(mod, half)] = wt
            return tiles

        def emit_A(blk, xTt, pre=None):
            gat = {}
            for mod in MODS:
                gat[mod] = p_gat.tile([128, NXC * LB], f32r, name="gat", tag="gat")
            for c in range(NXC):
                pp = {}
                for mod in MODS:
                    for half, m in (("x", c), ("w", NXC + c)):
                        if c == 0 and pre is not None:
                            wt = pre[(mod, half)]
                        else:
                            wt = p_win.tile([128, DIM], f32r, name="win", tag="win")
                            nc.sync.dma_start(
                                wt[:].rearrange("p (kc m) -> p kc m", kc=NKD),
                                win_dram[mod][m])
                        ps = p_ps.tile([128, LB], f32, name="ps", tag="ps")
                        for kc in range(NKD):
                            nc.tensor.matmul(
                                ps[:],
                                lhsT=wt[:, kc * 128:(kc + 1) * 128],
                                rhs=xTt[mod][:, kc * LB:(kc + 1) * LB],
                                start=(kc == 0), stop=(kc == NKD - 1))
                        pp[(mod, half)] = ps
                sv = {}
                for mod in MODS:
                    s = p_sv.tile([128, LB], f32, name="sv", tag="sv")
                    nc.scalar.activation(s[:], pp[(mod, "w")][:], AF.Silu)
                    sv[mod] = s
                for mi, (mod, other) in enumerate((("a", "v"), ("v", "a"))):
                    if conv_mode == "pe":
                        cm = "pe"
                    elif conv_mode == "dve":
                        cm = "dve"
                    elif conv_mode == "gps":
                        cm = "gps"
                    elif conv_mode == "dve+gps":
                        cm = "dve" if mi == 0 else "gps"
                    elif conv_mode == "pe+dve":
                        cm = "pe" if mi == 0 else "dve"
                    else:
                        raise ValueError(conv_mode)
                    axp = p_axp.tile([128, LB + 3], f32r, name="axp", tag="axp")
                    nc.vector.tensor_copy(axp[:, 0:3],
                                          hist[mod][:, c * 3:(c + 1) * 3])
                    nc.scalar.copy(axp[:, 3:LB + 3], pp[(mod, "x")][:])
                    nc.vector.tensor_copy(hist[mod][:, c * 3:(c + 1) * 3],
                                          axp[:, LB:LB + 3])
                    if cm == "pe":
                        cp = p_ps.tile([128, LB], f32, name="ps", tag="ps")
                        for t in range(4):
                            dg = p_diag.tile([128, 128], f32r, name="diag", tag="diag")
                            nc.vector.tensor_scalar(
                                dg[:], identity[:],
                                cw_sb[mod][:, c * 4 + t: c * 4 + t + 1],
                                None, AOP.mult)
                            nc.tensor.matmul(cp[:], lhsT=dg[:],
                                             rhs=axp[:, t:t + LB],
                                             start=(t == 0), stop=(t == 3))
                        conv_src = cp
                    else:
                        eng = nc.vector if cm == "dve" else nc.gpsimd
                        acc = p_sv.tile([128, LB], f32, name="convacc", tag="convacc")
                        eng.tensor_scalar(
                            acc[:], axp[:, 0:LB],
                            cw_sb[mod][:, c * 4: c * 4 + 1],
                            cbc_sb[mod][:, c:c + 1], AOP.mult, AOP.add)
                        for t in range(1, 4):
                            acc2 = p_sv.tile([128, LB], f32, name="convacc", tag="convacc")
                            eng.scalar_tensor_tensor(
                                acc2[:], axp[:, t:t + LB],
                                cw_sb[mod][:, c * 4 + t: c * 4 + t + 1],
                                acc[:], AOP.mult, AOP.add)
                            acc = acc2
                        conv_src = acc
                    if cm == "pe":
                        nc.vector.scalar_tensor_tensor(
                            gat[mod][:, c * LB:(c + 1) * LB], conv_src[:],
                            cbc_sb[mod][:, c:c + 1], sv[other][:],
                            AOP.add, AOP.mult)
                    else:
                        nc.vector.tensor_mul(gat[mod][:, c * LB:(c + 1) * LB],
                                             conv_src[:], sv[other][:])

            return gat

        def emit_B(blk, gat):
            l0 = blk * LB
            for mod in MODS:
                yoff = 0 if mod == "a" else L
                for n in range(NN):
                    po = [p_ps.tile([128, 512], f32, name="ps", tag="ps")
                          for _ in range(NLT)]
                    for c2 in range(NXC):
                        w = p_wout.tile([128, 512], f32r, name="wout", tag="wout")
                        nc.sync.dma_start(w[:], wout_dram[mod][c2, n])
                        for mt in range(NLT):
                            nc.tensor.matmul(
                                po[mt][:],
                                lhsT=gat[mod][:, c2 * LB + mt * 128:
                                              c2 * LB + (mt + 1) * 128],
                                rhs=w[:],
                                start=(c2 == 0), stop=(c2 == NXC - 1))
                    for mt in range(NLT):
                        rt = p_res.tile([128, 512], f32, name="res", tag="res")
                        nc.sync.dma_start(
                            rt[:], x_dram[mod][l0 + mt * 128: l0 + (mt + 1) * 128,
                                               n * 512:(n + 1) * 512])
                        yt = p_yout.tile([128, 512], f32, name="yout", tag="yout")
                        nc.vector.tensor_add(yt[:], po[mt][:], rt[:])
                        nc.sync.dma_start(
                            y[yoff + l0 + mt * 128: yoff + l0 + (mt + 1) * 128,
                              n * 512:(n + 1) * 512], yt[:])

        setup_conv_state()
        for rep in range(repeat):
            pre = None
            for blk in range(NB):
                xTt_cur = emit_T(blk)
                gat = emit_A(blk, xTt_cur, pre)
                pre = prefetch_w(blk + 1) if blk + 1 < NB else None
                emit_B(blk, gat)

    nc.finalize()
    return nc


def _get_nc(conv_mode=None, repeat=1):
    key = ("nc", conv_mode or CONV_MODE, repeat)
    if key not in _cache:
        _cache[key] = _build_nc(conv_mode, repeat)
    return _cache[key]


def _prep_weights(inputs):
    f = np.float32
    a_in = np.asarray(inputs["a_in_w"], f) * np.asarray(inputs["a_norm_w"], f)[None, :]
    v_in = np.asarray(inputs["v_in_w"], f) * np.asarray(inputs["v_norm_w"], f)[None, :]

    def pack_in(w):  # [2*INNER, DIM] -> [32, 128, 8, 128]: m-tile x [d%128, d//128, e]
        t = w.T.reshape(NKD, 128, 2 * NXC, 128)   # [kc, p, m, e]
        return np.ascontiguousarray(t.transpose(2, 1, 0, 3))

    def pack_out(w):  # [DIM, INNER] -> [16, 2, 128, 512] (e-chunk, d-tile)
        return np.ascontiguousarray(
            w.T.reshape(NXC, 128, NN, 512).transpose(0, 2, 1, 3))

    def pack_cw(w):  # [INNER, 1, 4] -> [128, 64]
        return np.ascontiguousarray(
            np.asarray(w, f)[:, 0, :].reshape(NXC, 128, 4)
            .transpose(1, 0, 2).reshape(128, NXC * 4))

    return {
        "wina": pack_in(a_in),
        "winv": pack_in(v_in),
        "wouta": pack_out(np.asarray(inputs["a_out_w"], f)),
        "woutv": pack_out(np.asarray(inputs["v_out_w"], f)),
        "cwa": pack_cw(inputs["a_conv_w"]),
        "cwv": pack_cw(inputs["v_conv_w"]),
        "cbca": np.ascontiguousarray(
            np.asarray(inputs["a_conv_b"], f).reshape(NXC, 128).T),
        "cbcv": np.ascontiguousarray(
            np.asarray(inputs["v_conv_b"], f).reshape(NXC, 128).T),
    }


def _make_in_maps(inputs):
    shared = _prep_weights(inputs)
    audio = np.ascontiguousarray(np.asarray(inputs["audio"], np.float32))
    visual = np.ascontiguousarray(np.asarray(inputs["visual"], np.float32))
    in_maps = []
    for b in range(B):
        m = dict(shared)
        m["xa"] = audio[b]
        m["xv"] = visual[b]
        in_maps.append(m)
    return in_maps


def _run(in_maps):
    from concourse.bass_utils import run_bass_kernel_spmd
    nc = _get_nc()
    res = run_bass_kernel_spmd(nc, in_maps, core_ids=list(range(NCORES)))
    return np.stack([res.results[b]["y"] for b in range(B)], axis=0)


def kernel(**inputs) -> np.ndarray:
    return _run(_make_in_maps(inputs))


# revision 31
# speedup vs baseline: 1.9860x; 1.0415x over previous
"""AV temporal gated-conv MLP block for Trainium2 (8 NeuronCores, Bass/Tile).

Per-core strategy: pure data parallelism over the batch (B=8 -> 1 batch
element per core, both modalities on the same core since the gating couples
them). No collectives. Per core, loop over 4 l-blocks of 512 tokens:

  T: rms-norm in natural [l, d] layout (ACT Square+accum_out -> Sqrt -> DVE
     reciprocal -> tensor_scalar scale), then PE-transpose x_n to [d, l].
  A: in_proj as fp32r matmuls producing [e, l] chunks; causal depthwise conv
     on the x-half via 4 diagonal matmuls + K=1 bias matmul accumulated in
     PSUM (diag built on the fly: identity x per-partition conv weight);
     silu (ACT) on the w-half straight out of PSUM; cross-modal gate on DVE.
  B: out_proj fp32r matmuls [l, d] + residual add + DMA out.

Weights are host-side packed: norm weights folded into in_proj, transposed to
[d, e] / [e, d] m-tile-contiguous layouts so every weight DMA is contiguous.
"""
import sys

if "/opt/trn_rl_repo" not in sys.path:
    sys.path.insert(0, "/opt/trn_rl_repo")

import numpy as np

DIM = 1024
INNER = 2048
L = 2048
B = 8
NCORES = 8
EPS = 1e-5
LB = 512              # l-block (tokens per block)
NB = L // LB          # 4 blocks
NXC = INNER // 128    # 16 x-half e-chunks per modality
NKD = DIM // 128      # 8 contraction chunks for in_proj
NLT = LB // 128       # 4 l-tiles per block
NN = DIM // 512       # 2 out_proj n-tiles
CONV_MODE = "dve"  # "pe" | "dve" | "gps" | "dve+gps" | "pe+dve"

_cache = {}


def _build_nc(conv_mode=None, repeat=1):
    conv_mode = conv_mode or CONV_MODE
    from contextlib import ExitStack

    import concourse.bass as bass
    import concourse.tile as tile
    from concourse import bacc, mybir
    from concourse.masks import make_identity

    dt = mybir.dt
    f32 = dt.float32
    f32r = dt.float32r
    bf16 = dt.bfloat16
    AOP = mybir.AluOpType
    AF = mybir.ActivationFunctionType

    nc = bacc.Bacc("TRN2", target_bir_lowering=False, debug=False,
                   num_devices=NCORES)

    x_dram = {
        "a": nc.dram_tensor("xa", [L, DIM], f32, kind="ExternalInput").ap(),
        "v": nc.dram_tensor("xv", [L, DIM], f32, kind="ExternalInput").ap(),
    }
    win_dram = {
        "a": nc.dram_tensor("wina", [2 * NXC, 128, NKD, 128], f32r,
                            kind="ExternalInput").ap(),
        "v": nc.dram_tensor("winv", [2 * NXC, 128, NKD, 128], f32r,
                            kind="ExternalInput").ap(),
    }
    wout_dram = {
        "a": nc.dram_tensor("wouta", [NXC, NN, 128, 512], f32r,
                            kind="ExternalInput").ap(),
        "v": nc.dram_tensor("woutv", [NXC, NN, 128, 512], f32r,
                            kind="ExternalInput").ap(),
    }
    cw_dram = {
        "a": nc.dram_tensor("cwa", [128, NXC * 4], f32, kind="ExternalInput").ap(),
        "v": nc.dram_tensor("cwv", [128, NXC * 4], f32, kind="ExternalInput").ap(),
    }
    cbc_dram = {
        "a": nc.dram_tensor("cbca", [128, NXC], f32, kind="ExternalInput").ap(),
        "v": nc.dram_tensor("cbcv", [128, NXC], f32, kind="ExternalInput").ap(),
    }
    y = nc.dram_tensor("y", [2 * L, DIM], f32, kind="ExternalOutput").ap()

    MODS = ("a", "v")

    with tile.TileContext(nc) as tc, ExitStack() as ctx:
        sing = ctx.enter_context(tc.tile_pool(name="sing", bufs=1))
        p_xT = ctx.enter_context(tc.tile_pool(name="xT", bufs=2))
        p_gat = ctx.enter_context(tc.tile_pool(name="gat", bufs=2))
        p_xin = ctx.enter_context(tc.tile_pool(name="xin", bufs=3))
        p_xn = ctx.enter_context(tc.tile_pool(name="xn", bufs=5))
        p_stat = ctx.enter_context(tc.tile_pool(name="stat", bufs=8))
        p_win = ctx.enter_context(tc.tile_pool(name="win", bufs=6))
        p_wout = ctx.enter_context(tc.tile_pool(name="wout", bufs=4))
        p_axp = ctx.enter_context(tc.tile_pool(name="axp", bufs=4))
        p_sv = ctx.enter_context(tc.tile_pool(name="sv", bufs=3))
        p_diag = ctx.enter_context(tc.tile_pool(name="diag", bufs=4))
        p_res = ctx.enter_context(tc.tile_pool(name="res", bufs=4))
        p_yout = ctx.enter_context(tc.tile_pool(name="yout", bufs=4))
        p_ps = ctx.enter_context(
            tc.tile_pool(name="ps", bufs=8, space=bass.MemorySpace.PSUM))

        ident_f32 = sing.tile([128, 128], f32, name="ident_f32", tag="ident_f32")
        make_identity(nc, ident_f32[:])
        identity = sing.tile([128, 128], f32r, name="identity", tag="identity")
        nc.vector.tensor_copy(identity[:], ident_f32[:])
        epscol = sing.tile([128, 1], f32, name="epscol", tag="epscol")
        nc.vector.memset(epscol[:], EPS)

        cw_sb, cbc_sb, hist = {}, {}, {}

        def setup_conv_state():
            for mod in MODS:
                cw_sb[mod] = sing.tile([128, NXC * 4], f32, name=f"cw_{mod}",
                                       tag=f"cw_{mod}")
                nc.sync.dma_start(cw_sb[mod][:], cw_dram[mod][:])
                cbc_sb[mod] = sing.tile([128, NXC], f32, name=f"cbc_{mod}",
                                        tag=f"cbc_{mod}")
                nc.sync.dma_start(cbc_sb[mod][:], cbc_dram[mod][:])
                hist[mod] = sing.tile([128, NXC * 3], f32, name=f"hist_{mod}",
                                      tag=f"hist_{mod}")
                nc.vector.memset(hist[mod][:], 0.0)

        def emit_T(blk):
            l0 = blk * LB
            xTt = {}
            for mod in MODS:
                xTt[mod] = p_xT.tile([128, NKD * LB], f32r, name="xT", tag="xT")
                for lt in range(NLT):
                    xt = p_xin.tile([128, DIM], f32, name="xin", tag="xin")
                    nc.sync.dma_start(
                        xt[:], x_dram[mod][l0 + lt * 128: l0 + (lt + 1) * 128, :])
                    xnt = p_xn.tile([128, DIM], f32r, name="xn", tag="xn")
                    ssum = p_stat.tile([128, 1], f32, name="ssum", tag="ssum")
                    nc.scalar.activation(xnt[:], xt[:], AF.Square,
                                         accum_out=ssum[:])
                    sroot = p_stat.tile([128, 1], f32, name="sroot", tag="sroot")
                    nc.scalar.activation(sroot[:], ssum[:], AF.Sqrt,
                                         bias=epscol[:], scale=1.0 / DIM)
                    sinv = p_stat.tile([128, 1], f32, name="sinv", tag="sinv")
                    nc.vector.reciprocal(sinv[:], sroot[:])
                    nc.vector.tensor_scalar(xnt[:], xt[:], sinv[:], None,
                                            AOP.mult)
                    for g in range(2):
                        pt = p_ps.tile([128, 512], f32, name="ps", tag="ps")
                        for j in range(4):
                            dc = g * 4 + j
                            nc.tensor.matmul(
                                pt[:, j * 128:(j + 1) * 128].bitcast(f32r),
                                lhsT=xnt[:, dc * 128:(dc + 1) * 128],
                                rhs=identity[:],
                                is_transpose=True, skip_group_check=True)
                        dst = xTt[mod].rearrange("p (dc l) -> p dc l", dc=NKD)[
                            :, g * 4:(g + 1) * 4, lt * 128:(lt + 1) * 128]
                        src = pt[:].rearrange("p (j l) -> p j l", j=4)
                        if g == 0:
                            nc.vector.tensor_copy(dst, src)
                        else:
                            nc.scalar.copy(dst, src)

            return xTt

        def prefetch_w(blk):
            tiles = {}
            for mod in MODS:
                for half, m in (("x", 0), ("w", NXC)):
                    wt = p_win.tile([128, DIM], f32r, name="win", tag="win")
                    nc.sync.dma_start(
                        wt[:].rearrange("p (kc m) -> p kc m", kc=NKD),
                        win_dram[mod][m])
                    tiles[(mod, half)] = wt
            return tiles

        def emit_A(blk, xTt, pre=None):
            gat = {}
            for mod in MODS:
                gat[mod] = p_gat.tile([128, NXC * LB], f32r, name="gat", tag="gat")
            for c in range(NXC):
                pp = {}
                for mod in MODS:
                    for half, m in (("x", c), ("w", NXC + c)):
                        if c == 0 and pre is not None:
                            wt = pre[(mod, half)]
                        else:
                            wt = p_win.tile([128, DIM], f32r, name="win", tag="win")
                            nc.sync.dma_start(
                                wt[:].rearrange("p (kc m) -> p kc m", kc=NKD),
                                win_dram[mod][m])
                        ps = p_ps.tile([128, LB], f32, name="ps", tag="ps")
                        for kc in range(NKD):
                            nc.tensor.matmul(
                                ps[:],
                                lhsT=wt[:, kc * 128:(kc + 1) * 128],
                                rhs=xTt[mod][:, kc * LB:(kc + 1) * LB],
                                start=(kc == 0), stop=(kc == NKD - 1))
                        pp[(mod, half)] = ps
                sv = {}
                for mod in MODS:
                    s = p_sv.tile([128, LB], f32, name="sv", tag="sv")
                    nc.scalar.activation(s[:], pp[(mod, "w")][:], AF.Silu)
                    sv[mod] = s
                for mi, (mod, other) in enumerate((("a", "v"), ("v", "a"))):
                    if conv_mode == "pe":
                        cm = "pe"
                    elif conv_mode == "dve":
                        cm = "dve"
                    elif conv_mode == "gps":
                        cm = "gps"
                    elif conv_mode == "dve+gps":
                        cm = "dve" if mi == 0 else "gps"
                    elif conv_mode == "pe+dve":
                        cm = "pe" if mi == 0 else "dve"
                    else:
                        raise ValueError(conv_mode)
                    axp = p_axp.tile([128, LB + 3], f32r, name="axp", tag="axp")
                    nc.vector.tensor_copy(axp[:, 0:3],
                                          hist[mod][:, c * 3:(c + 1) * 3])
                    nc.scalar.copy(axp[:, 3:LB + 3], pp[(mod, "x")][:])
                    nc.vector.tensor_copy(hist[mod][:, c * 3:(c + 1) * 3],
                                          axp[:, LB:LB + 3])
                    if cm == "pe":
                        cp = p_ps.tile([128, LB], f32, name="ps", tag="ps")
                        for t in range(4):
                            dg = p_diag.tile([128, 128], f32r, name="diag", tag="diag")
                            nc.vector.tensor_scalar(
                                dg[:], identity[:],
                                cw_sb[mod][:, c * 4 + t: c * 4 + t + 1],
                                None, AOP.mult)
                            nc.tensor.matmul(cp[:], lhsT=dg[:],
                                             rhs=axp[:, t:t + LB],
                                             start=(t == 0), stop=(t == 3))
                        conv_src = cp
                    else:
                        eng = nc.vector if cm == "dve" else nc.gpsimd
                        acc = p_sv.tile([128, LB], f32, name="convacc", tag="convacc")
                        eng.tensor_scalar(
                            acc[:], axp[:, 0:LB],
                            cw_sb[mod][:, c * 4: c * 4 + 1],
                            cbc_sb[mod][:, c:c + 1], AOP.mult, AOP.add)
                        for t in range(1, 4):
                            acc2 = p_sv.tile([128, LB], f32, name="convacc", tag="convacc")
                            eng.scalar_tensor_tensor(
                                acc2[:], axp[:, t:t + LB],
                                cw_sb[mod][:, c * 4 + t: c * 4 + t + 1],
                                acc[:], AOP.mult, AOP.add)
                            acc = acc2
                        conv_src = acc
                    if cm == "pe":
                        nc.vector.scalar_tensor_tensor(
                            gat[mod][:, c * LB:(c + 1) * LB], conv_src[:],
                            cbc_sb[mod][:, c:c + 1], sv[other][:],
                            AOP.add, AOP.mult)
                    else:
                        nc.vector.tensor_mul(gat[mod][:, c * LB:(c + 1) * LB],
                                             conv_src[:], sv[other][:])

            return gat

        def emit_B(blk, gat):
            l0 = blk * LB
            for mod in MODS:
                yoff = 0 if mod == "a" else L
                for n in range(NN):
                    po = [p_ps.tile([128, 512], f32, name="ps", tag="ps")
                          for _ in range(NLT)]
                    for c2 in range(NXC):
                        w = p_wout.tile([128, 512], f32r, name="wout", tag="wout")
                        nc.sync.dma_start(w[:], wout_dram[mod][c2, n])
                        for mt in range(NLT):
                            nc.tensor.matmul(
                                po[mt][:],
                                lhsT=gat[mod][:, c2 * LB + mt * 128:
                                              c2 * LB + (mt + 1) * 128],
                                rhs=w[:],
                                start=(c2 == 0), stop=(c2 == NXC - 1))
                    for mt in range(NLT):
                        rt = p_res.tile([128, 512], f32, name="res", tag="res")
                        nc.sync.dma_start(
                            rt[:], x_dram[mod][l0 + mt * 128: l0 + (mt + 1) * 128,
                                               n * 512:(n + 1) * 512])
                        yt = p_yout.tile([128, 512], f32, name="yout", tag="yout")
                        nc.vector.tensor_add(yt[:], po[mt][:], rt[:])
                        nc.sync.dma_start(
                            y[yoff + l0 + mt * 128: yoff + l0 + (mt + 1) * 128,
                              n * 512:(n + 1) * 512], yt[:])

        setup_conv_state()
        for rep in range(repeat):
            pre = None
            for blk in range(NB):
                xTt_cur = emit_T(blk)
                gat = emit_A(blk, xTt_cur, pre)
                pre = prefetch_w(blk + 1) if blk + 1 < NB else None
                emit_B(blk, gat)

    nc.finalize()
    return nc


def _get_nc(conv_mode=None, repeat=1):
    key = ("nc", conv_mode or CONV_MODE, repeat)
    if key not in _cache:
        _cache[key] = _build_nc(conv_mode, repeat)
    return _cache[key]


def _prep_weights(inputs):
    f = np.float32
    a_in = np.asarray(inputs["a_in_w"], f) * np.asarray(inputs["a_norm_w"], f)[None, :]
    v_in = np.asarray(inputs["v_in_w"], f) * np.asarray(inputs["v_norm_w"], f)[None, :]

    def pack_in(w):  # [2*INNER, DIM] -> [32, 128, 8, 128]: m-tile x [d%128, d//128, e]
        t = w.T.reshape(NKD, 128, 2 * NXC, 128)   # [kc, p, m, e]
        return np.ascontiguousarray(t.transpose(2, 1, 0, 3))

    def pack_out(w):  # [DIM, INNER] -> [16, 2, 128, 512] (e-chunk, d-tile)
        return np.ascontiguousarray(
            w.T.reshape(NXC, 128, NN, 512).transpose(0, 2, 1, 3))

    def pack_cw(w):  # [INNER, 1, 4] -> [128, 64]
        return np.ascontiguousarray(
            np.asarray(w, f)[:, 0, :].reshape(NXC, 128, 4)
            .transpose(1, 0, 2).reshape(128, NXC * 4))

    return {
        "wina": pack_in(a_in),
        "winv": pack_in(v_in),
        "wouta": pack_out(np.asarray(inputs["a_out_w"], f)),
        "woutv": pack_out(np.asarray(inputs["v_out_w"], f)),
        "cwa": pack_cw(inputs["a_conv_w"]),
        "cwv": pack_cw(inputs["v_conv_w"]),
        "cbca": np.ascontiguousarray(
            np.asarray(inputs["a_conv_b"], f).reshape(NXC, 128).T),
        "cbcv": np.ascontiguousarray(
            np.asarray(inputs["v_conv_b"], f).reshape(NXC, 128).T),
    }


def _make_in_maps(inputs):
    shared = _prep_weights(inputs)
    audio = np.ascontiguousarray(np.asarray(inputs["audio"], np.float32))
    visual = np.ascontiguousarray(np.asarray(inputs["visual"], np.float32))
    in_maps = []
    for b in range(B):
        m = dict(shared)
        m["xa"] = audio[b]
        m["xv"] = visual[b]
        in_maps.append(m)
    return in_maps


def _run(in_maps):
    from concourse.bass_utils import run_bass_kernel_spmd
    nc = _get_nc()
    res = run_bass_kernel_spmd(nc, in_maps, core_ids=list(range(NCORES)))
    return np.stack([res.results[b]["y"] for b in range(B)], axis=0)


def kernel(**inputs) -> np.ndarray:
    return _run(_make_in_maps(inputs))
